# revision 2
# baseline (speedup 1.0000x reference)
"""GATReduce Trainium2 kernel (8-core SPMD, node-major layout).

Reference computation (per node n, head h, feature f):
    a[n,d,h] = a1[n,h] + a2[n,d,h]
    e = softmax_d(leaky_relu(a, 0.01))
    out[n,h,f] = sum_d e[n,d,h] * ft[n,d,h,f]

Shapes: N=16384 nodes, D=32 mailbox, H=8 heads, F=64 features. fp32.
Data-parallel over nodes: 2048 nodes per core, no communication.

Layout: partition p = node within a 128-node tile.  Everything for one
node (a2 row, softmax, ft slab, output row) lives on one partition:

  * ft HBM lines are fully contiguous per node (64 KB); the tile DMA is
    split into 4 column chunks (16 KB lines, 128 descriptors each) so
    each chunk's multiply can start as soon as its quarter lands.
    The (d,nl)-partition layout this replaced needed 2 KB lines (1024
    descriptors/tile) for ft and 32 B lines (4096/tile!) for a2; here
    a2 is one 128 x 1 KB DMA and out one 128 x 2 KB DMA per tile.
  * ft is cast fp32->bf16 DURING the DMA (SWDGE/gpsimd, free on the
    DMA datapath -- measured identical BW to a plain fp32 load): halves
    SBUF footprint and enables the DVE 2x_1P tensor_tensor mode.
  * softmax over d is a free-dim reduction: DVE add (a1 broadcast) ->
    DVE leaky-relu -> ScalarE Exp -> DVE tensor_reduce(sum over d) ->
    DVE reciprocal.  No transposes / onehot / repmat matmuls.
  * q = e (x) ft on the DVE in bf16 at 2 elem/cyc/lane (2x_1P): e is
    materialized as PAIRS (ScalarE copy duplicating each value into
    [., 2]) so the broadcast AP has innermost step 1 -- a plain
    stride-0 broadcast drops the DVE to 1x and costs +240 us/iter.
  * sum over d on the PE: 32 accumulating identity matmuls per tile
    into one PSUM bank (start at d=0, stop at d=31), bf16 moving
    1 cyc/col, ~110 us/core busy.
  * out = psum * (1/s): ScalarE drains PSUM->SBUF, DVE multiplies by
    the fp32 softmax reciprocal, HWDGE-scalar queue stores.
  * All softmax-side pools use bufs=4: with 16 tiles/iteration, buffer
    0's last user is tile 12, so the next For_i iteration's tile 0
    never waits on the previous iteration's tile 14/15 (bufs=2/3 stall
    the ft stream at every loop back-edge).

Engine budget per core-iter (cost model): DMA ~134 MB ft + 6 MB a2/out
at the HBM-per-NC limit; DVE ~150 us busy; PE ~110 us; ScalarE ~40 us;
GPSIMD Q7 SWDGE descriptor emission ~200 us (hidden under transfers).
The kernel is HBM-bandwidth-bound: measured 318 us/iter on a quiet
device (~440 GB/s effective), 380-440 us under co-tenant HBM load (the
~358 GB/s fair-share regime), vs 527-586 us for the previous
(d,nl)-layout kernel whose fp32 1x DVE multiply (273 us) and 90k
descriptors/iter could not stay hidden under the DMA.

Verified rel err vs reference: 7.0e-3 (bf16 ft+e rounding; budget 2e-2).
"""

import numpy as np

import concourse.bacc as bacc
import concourse.bass as bass
import concourse.tile as tile
from concourse import mybir
from concourse.bass_utils import run_bass_kernel_spmd

N_CORES = 8
N, D, H, F = 16384, 32, 8, 64
N_PER_CORE = N // N_CORES  # 2048
TILE_N = 128  # nodes per tile = partitions
DH = D * H  # 256
HF = H * F  # 512
ELEMS = D * H * F  # 16384 elems = 64 KB fp32 per node
CH = 8  # d-chunk for the multiply (4 chunks per tile)
NEG_SLOPE = 0.01

_FP = mybir.dt.float32
_BF = mybir.dt.bfloat16


def build(
    n_per_core: int = N_PER_CORE,
    reps: int = 1,
    loop_iters: int | None = None,
    internal_ft: bool = False,
) -> bass.Bass:
    assert n_per_core % TILE_N == 0
    n_tiles = n_per_core // TILE_N

    nc = bacc.Bacc(
        "TRN2", target_bir_lowering=False, debug=False, num_devices=N_CORES
    )
    a1_h = nc.declare_dram_parameter("a1", [n_per_core, H], _FP, isOutput=False)
    a2_h = nc.declare_dram_parameter("a2", [n_per_core, DH], _FP, isOutput=False)
    if internal_ft:
        ft_h = nc.dram_tensor("ft_int", [n_per_core, ELEMS], _FP)
    else:
        ft_h = nc.declare_dram_parameter(
            "ft", [n_per_core, ELEMS], _FP, isOutput=False
        )
    ident_h = nc.declare_dram_parameter("ident", [128, 128], _BF, isOutput=False)
    out_h = nc.declare_dram_parameter("out", [n_per_core, HF], _FP, isOutput=True)

    with tile.TileContext(nc) as tc:
        import contextlib

        with contextlib.ExitStack() as ctx:
            consts = ctx.enter_context(tc.tile_pool(name="consts", bufs=1))
            a2p = ctx.enter_context(tc.tile_pool(name="a2p", bufs=4))
            smx = ctx.enter_context(tc.tile_pool(name="smx", bufs=4))
            e2p = ctx.enter_context(tc.tile_pool(name="e2p", bufs=4))
            rp = ctx.enter_context(tc.tile_pool(name="rp", bufs=4))
            ftp = ctx.enter_context(tc.tile_pool(name="ftp", bufs=4))
            qp = ctx.enter_context(tc.tile_pool(name="qp", bufs=2))
            pso = ctx.enter_context(tc.tile_pool(name="pso", bufs=2, space="PSUM"))
            outp = ctx.enter_context(tc.tile_pool(name="outp", bufs=4))

            ident_t = consts.tile([128, 128], _BF)
            nc.sync.dma_start(out=ident_t[:], in_=ident_h[:])
            # a1 in node-major layout: [p, t, h]; node = t*128 + p
            a1_all = consts.tile([128, n_tiles, H], _FP)
            nc.sync.dma_start(
                out=a1_all[:],
                in_=a1_h[:].rearrange("(t p) h -> p t h", p=TILE_N),
            )

            if loop_iters is not None:
                rep_iter = [None]
                loop_cm = tc.For_i(0, loop_iters, 1)
            else:
                rep_iter = list(range(reps))
                loop_cm = contextlib.nullcontext()

            def emit_tile(t):
                n0 = t * TILE_N

                # ---- ft load (cast fp32->bf16 during DMA), 16 KB lines,
                # one DMA per d-chunk so multiplies start on first arrival
                ft_t = ftp.tile([128, ELEMS], _BF)
                CW = CH * HF  # 4096 elems per chunk
                for c in range(D // CH):
                    nc.gpsimd.dma_start(
                        out=ft_t[:, c * CW : (c + 1) * CW],
                        in_=ft_h[n0 : n0 + TILE_N, c * CW : (c + 1) * CW],
                    )

                # ---- softmax over d (free dim)
                a2_t = a2p.tile([128, D, H], _FP)
                nc.sync.dma_start(
                    out=a2_t[:],
                    in_=a2_h[n0 : n0 + TILE_N].rearrange("n (d h) -> n d h", h=H),
                )
                p_t = smx.tile([128, D, H], _FP, tag="p_t")
                nc.vector.tensor_tensor(
                    out=p_t[:],
                    in0=a2_t[:],
                    in1=a1_all[:, t].unsqueeze(1).to_broadcast((128, D, H)),
                    op=mybir.AluOpType.add,
                )
                nc.vector.scalar_tensor_tensor(
                    out=p_t[:],
                    in0=p_t[:],
                    scalar=NEG_SLOPE,
                    in1=p_t[:],
                    op0=mybir.AluOpType.mult,
                    op1=mybir.AluOpType.max,
                )
                nc.scalar.activation(
                    out=p_t[:], in_=p_t[:], func=mybir.ActivationFunctionType.Exp
                )
                # unnormalized e as bf16 pairs (ScalarE; keeps DVE lean)
                p2_t = e2p.tile([128, DH, 2], _BF, tag="p2_t")
                nc.scalar.copy(
                    out=p2_t[:],
                    in_=p_t[:]
                    .rearrange("p d h -> p (d h)")
                    .unsqueeze(-1)
                    .to_broadcast((128, DH, 2)),
                )
                # softmax denominator (fp32) and reciprocal
                s_t = rp.tile([128, H], _FP, tag="s_t")
                nc.vector.tensor_reduce(
                    out=s_t[:],
                    in_=p_t[:].rearrange("p d h -> p h d"),
                    axis=mybir.AxisListType.X,
                    op=mybir.AluOpType.add,
                )
                r_t = rp.tile([128, H], _FP, tag="r_t")
                nc.vector.reciprocal(out=r_t[:], in_=s_t[:])

                # ---- q = p (x) ft (bf16, 2x_1P) + PE accumulate over d
                ps = pso.tile([128, HF], _FP)
                for c in range(D // CH):
                    q_t = qp.tile([128, CH * HF], _BF)
                    nc.vector.tensor_tensor(
                        out=q_t[:].rearrange(
                            "p (x f2 two) -> p x f2 two", f2=F // 2, two=2
                        ),
                        in0=ft_t[:, c * CW : (c + 1) * CW].rearrange(
                            "p (x f2 two) -> p x f2 two", f2=F // 2, two=2
                        ),
                        in1=p2_t[:, c * CH * H : (c + 1) * CH * H]
                        .unsqueeze(2)
                        .to_broadcast((128, CH * H, F // 2, 2)),
                        op=mybir.AluOpType.mult,
                    )
                    for k in range(CH):
                        d = c * CH + k
                        nc.tensor.matmul(
                            ps[:],
                            ident_t[:],
                            q_t[:, k * HF : (k + 1) * HF],
                            start=(d == 0),
                            stop=(d == D - 1),
                        )

                # ---- drain + normalize + store
                o_t = outp.tile([128, HF], _FP)
                nc.scalar.copy(out=o_t[:], in_=ps[:])
                nc.vector.tensor_tensor(
                    out=o_t[:].rearrange("p (h f) -> p h f", f=F),
                    in0=o_t[:].rearrange("p (h f) -> p h f", f=F),
                    in1=r_t[:].unsqueeze(-1).to_broadcast((128, H, F)),
                    op=mybir.AluOpType.mult,
                )
                nc.scalar.dma_start(out=out_h[n0 : n0 + TILE_N], in_=o_t[:])

            with loop_cm:
                for _ in rep_iter:
                    for t in range(n_tiles):
                        emit_tile(t)

    nc.compile()
    return nc


def run(
    a1: np.ndarray,
    a2: np.ndarray,
    ft: np.ndarray,
    n_per_core: int = N_PER_CORE,
    reps: int = 1,
    nc: bass.Bass | None = None,
):
    import ml_dtypes

    if nc is None:
        nc = build(n_per_core, reps)
    ident = np.eye(128, dtype=ml_dtypes.bfloat16)
    ft_names = {
        a.memorylocations[0].name
        for a in nc.m.functions[0].allocations
        if getattr(a, "kind", None) == "ExternalInput"
    }
    in_maps = []
    for c in range(N_CORES):
        sl = slice(c * n_per_core, (c + 1) * n_per_core)
        m = {
            "a1": np.ascontiguousarray(a1[sl]),
            "a2": np.ascontiguousarray(a2[sl]).reshape(n_per_core, DH),
            "ident": ident,
        }
        if "ft" in ft_names:
            m["ft"] = np.ascontiguousarray(ft[sl]).reshape(n_per_core, ELEMS)
        in_maps.append(m)
    res = run_bass_kernel_spmd(nc, in_maps, list(range(N_CORES)))
    out = np.concatenate([res.results[c]["out"] for c in range(N_CORES)], axis=0)
    return out.reshape(-1, H, F)


def kernel(a1: np.ndarray, a2: np.ndarray, ft: np.ndarray) -> np.ndarray:
    a1 = np.asarray(a1, dtype=np.float32)
    a2 = np.asarray(a2, dtype=np.float32)
    ft = np.asarray(ft, dtype=np.float32)
    assert a1.shape == (N, H, 1) and a2.shape == (N, D, H, 1)
    assert ft.shape == (N, D, H, F)
    out = run(a1.reshape(N, H), a2.reshape(N, D, H), ft)
    return out.astype(np.float32)


# revision 3
# speedup vs baseline: 1.3597x; 1.3597x over previous
"""GATReduce Trainium2 kernel (8-core SPMD, node-major layout).

Reference computation (per node n, head h, feature f):
    a[n,d,h] = a1[n,h] + a2[n,d,h]
    e = softmax_d(leaky_relu(a, 0.01))
    out[n,h,f] = sum_d e[n,d,h] * ft[n,d,h,f]

Shapes: N=16384 nodes, D=32 mailbox, H=8 heads, F=64 features. fp32.
Data-parallel over nodes: 2048 nodes per core, no communication.

Layout: partition p = node within a 128-node tile.  Everything for one
node (a2 row, softmax, ft slab, output row) lives on one partition:

  * ft HBM lines are fully contiguous per node (64 KB); the tile DMA is
    split into 4 column chunks (16 KB lines, 128 descriptors each) so
    each chunk's multiply can start as soon as its quarter lands.
    The (d,nl)-partition layout this replaced needed 2 KB lines (1024
    descriptors/tile) for ft and 32 B lines (4096/tile!) for a2; here
    a2 is one 128 x 1 KB DMA and out one 128 x 2 KB DMA per tile.
  * ft is cast fp32->bf16 DURING the DMA (SWDGE/gpsimd, free on the
    DMA datapath -- measured identical BW to a plain fp32 load): halves
    SBUF footprint and enables the DVE 2x_1P tensor_tensor mode.
  * softmax over d is a free-dim reduction: DVE add (a1 broadcast) ->
    DVE leaky-relu -> ScalarE Exp -> DVE tensor_reduce(sum over d) ->
    DVE reciprocal.  No transposes / onehot / repmat matmuls.
  * q = e (x) ft on the DVE in bf16 at 2 elem/cyc/lane (2x_1P): e is
    materialized as PAIRS (ScalarE copy duplicating each value into
    [., 2]) so the broadcast AP has innermost step 1 -- a plain
    stride-0 broadcast drops the DVE to 1x and costs +240 us/iter.
  * sum over d on the PE: 32 accumulating identity matmuls per tile
    into one PSUM bank (start at d=0, stop at d=31), bf16 moving
    1 cyc/col, ~110 us/core busy.
  * out = psum * (1/s): one DVE tensor_tensor reads PSUM directly,
    multiplies by the fp32 softmax reciprocal and writes bf16 (halves
    the output HBM write; the host upcasts to fp32).
  * All softmax-side pools use bufs=4: with 16 tiles/iteration, buffer
    0's last user is tile 12, so the next For_i iteration's tile 0
    never waits on the previous iteration's tile 14/15 (bufs=2/3 stall
    the ft stream at every loop back-edge).

Engine budget per core-iter (cost model): DMA ~134 MB ft + 6 MB a2/out
at the HBM-per-NC limit; DVE ~150 us busy; PE ~110 us; ScalarE ~40 us;
GPSIMD Q7 SWDGE descriptor emission ~200 us (hidden under transfers).
The kernel is HBM-bandwidth-bound: measured 318 us/iter on a quiet
device (~440 GB/s effective), 380-440 us under co-tenant HBM load (the
~358 GB/s fair-share regime), vs 527-586 us for the previous
(d,nl)-layout kernel whose fp32 1x DVE multiply (273 us) and 90k
descriptors/iter could not stay hidden under the DMA.

Verified rel err vs reference: 7.0e-3 (bf16 ft+e rounding; budget 2e-2).
"""

import numpy as np

import concourse.bacc as bacc
import concourse.bass as bass
import concourse.tile as tile
from concourse import mybir
from concourse.bass_utils import run_bass_kernel_spmd

N_CORES = 8
N, D, H, F = 16384, 32, 8, 64
N_PER_CORE = N // N_CORES  # 2048
TILE_N = 128  # nodes per tile = partitions
DH = D * H  # 256
HF = H * F  # 512
ELEMS = D * H * F  # 16384 elems = 64 KB fp32 per node
CH = 8  # d-chunk for the multiply (4 chunks per tile)
NEG_SLOPE = 0.01

_FP = mybir.dt.float32
_BF = mybir.dt.bfloat16


def build(
    n_per_core: int = N_PER_CORE,
    reps: int = 1,
    loop_iters: int | None = None,
    internal_ft: bool = False,
) -> bass.Bass:
    assert n_per_core % TILE_N == 0
    n_tiles = n_per_core // TILE_N

    nc = bacc.Bacc(
        "TRN2", target_bir_lowering=False, debug=False, num_devices=N_CORES
    )
    a1_h = nc.declare_dram_parameter("a1", [n_per_core, H], _FP, isOutput=False)
    a2_h = nc.declare_dram_parameter("a2", [n_per_core, DH], _FP, isOutput=False)
    if internal_ft:
        ft_h = nc.dram_tensor("ft_int", [n_per_core, ELEMS], _FP)
    else:
        ft_h = nc.declare_dram_parameter(
            "ft", [n_per_core, ELEMS], _FP, isOutput=False
        )
    ident_h = nc.declare_dram_parameter("ident", [128, 128], _BF, isOutput=False)
    out_h = nc.declare_dram_parameter("out", [n_per_core, HF], _BF, isOutput=True)

    with tile.TileContext(nc) as tc:
        import contextlib

        with contextlib.ExitStack() as ctx:
            consts = ctx.enter_context(tc.tile_pool(name="consts", bufs=1))
            a2p = ctx.enter_context(tc.tile_pool(name="a2p", bufs=4))
            smx = ctx.enter_context(tc.tile_pool(name="smx", bufs=4))
            e2p = ctx.enter_context(tc.tile_pool(name="e2p", bufs=4))
            rp = ctx.enter_context(tc.tile_pool(name="rp", bufs=4))
            ftp = ctx.enter_context(tc.tile_pool(name="ftp", bufs=4))
            qp = ctx.enter_context(tc.tile_pool(name="qp", bufs=2))
            pso = ctx.enter_context(tc.tile_pool(name="pso", bufs=2, space="PSUM"))
            outp = ctx.enter_context(tc.tile_pool(name="outp", bufs=4))

            ident_t = consts.tile([128, 128], _BF)
            nc.sync.dma_start(out=ident_t[:], in_=ident_h[:])
            # a1 in node-major layout: [p, t, h]; node = t*128 + p
            a1_all = consts.tile([128, n_tiles, H], _FP)
            nc.sync.dma_start(
                out=a1_all[:],
                in_=a1_h[:].rearrange("(t p) h -> p t h", p=TILE_N),
            )

            if loop_iters is not None:
                rep_iter = [None]
                loop_cm = tc.For_i(0, loop_iters, 1)
            else:
                rep_iter = list(range(reps))
                loop_cm = contextlib.nullcontext()

            def emit_tile(t):
                n0 = t * TILE_N

                # ---- ft load (cast fp32->bf16 during DMA), 16 KB lines,
                # one DMA per d-chunk so multiplies start on first arrival
                ft_t = ftp.tile([128, ELEMS], _BF)
                CW = CH * HF  # 4096 elems per chunk
                for c in range(D // CH):
                    nc.gpsimd.dma_start(
                        out=ft_t[:, c * CW : (c + 1) * CW],
                        in_=ft_h[n0 : n0 + TILE_N, c * CW : (c + 1) * CW],
                    )

                # ---- softmax over d (free dim)
                a2_t = a2p.tile([128, D, H], _FP)
                nc.sync.dma_start(
                    out=a2_t[:],
                    in_=a2_h[n0 : n0 + TILE_N].rearrange("n (d h) -> n d h", h=H),
                )
                p_t = smx.tile([128, D, H], _FP, tag="p_t")
                nc.vector.tensor_tensor(
                    out=p_t[:],
                    in0=a2_t[:],
                    in1=a1_all[:, t].unsqueeze(1).to_broadcast((128, D, H)),
                    op=mybir.AluOpType.add,
                )
                nc.vector.scalar_tensor_tensor(
                    out=p_t[:],
                    in0=p_t[:],
                    scalar=NEG_SLOPE,
                    in1=p_t[:],
                    op0=mybir.AluOpType.mult,
                    op1=mybir.AluOpType.max,
                )
                nc.scalar.activation(
                    out=p_t[:], in_=p_t[:], func=mybir.ActivationFunctionType.Exp
                )
                # unnormalized e as bf16 pairs (ScalarE; keeps DVE lean)
                p2_t = e2p.tile([128, DH, 2], _BF, tag="p2_t")
                nc.scalar.copy(
                    out=p2_t[:],
                    in_=p_t[:]
                    .rearrange("p d h -> p (d h)")
                    .unsqueeze(-1)
                    .to_broadcast((128, DH, 2)),
                )
                # softmax denominator (fp32) and reciprocal
                s_t = rp.tile([128, H], _FP, tag="s_t")
                nc.vector.tensor_reduce(
                    out=s_t[:],
                    in_=p_t[:].rearrange("p d h -> p h d"),
                    axis=mybir.AxisListType.X,
                    op=mybir.AluOpType.add,
                )
                r_t = rp.tile([128, H], _FP, tag="r_t")
                nc.vector.reciprocal(out=r_t[:], in_=s_t[:])

                # ---- q = p (x) ft (bf16, 2x_1P) + PE accumulate over d
                ps = pso.tile([128, HF], _FP)
                for c in range(D // CH):
                    q_t = qp.tile([128, CH * HF], _BF)
                    nc.vector.tensor_tensor(
                        out=q_t[:].rearrange(
                            "p (x f2 two) -> p x f2 two", f2=F // 2, two=2
                        ),
                        in0=ft_t[:, c * CW : (c + 1) * CW].rearrange(
                            "p (x f2 two) -> p x f2 two", f2=F // 2, two=2
                        ),
                        in1=p2_t[:, c * CH * H : (c + 1) * CH * H]
                        .unsqueeze(2)
                        .to_broadcast((128, CH * H, F // 2, 2)),
                        op=mybir.AluOpType.mult,
                    )
                    for k in range(CH):
                        d = c * CH + k
                        nc.tensor.matmul(
                            ps[:],
                            ident_t[:],
                            q_t[:, k * HF : (k + 1) * HF],
                            start=(d == 0),
                            stop=(d == D - 1),
                        )

                # ---- drain + normalize + store (fused, bf16 out halves
                # the output HBM write; host upcasts to fp32)
                o_t = outp.tile([128, HF], _BF)
                nc.vector.tensor_tensor(
                    out=o_t[:].rearrange("p (h f) -> p h f", f=F),
                    in0=ps[:].rearrange("p (h f) -> p h f", f=F),
                    in1=r_t[:].unsqueeze(-1).to_broadcast((128, H, F)),
                    op=mybir.AluOpType.mult,
                )
                nc.scalar.dma_start(out=out_h[n0 : n0 + TILE_N], in_=o_t[:])

            with loop_cm:
                for _ in rep_iter:
                    for t in range(n_tiles):
                        emit_tile(t)

    nc.compile()
    return nc


def run(
    a1: np.ndarray,
    a2: np.ndarray,
    ft: np.ndarray,
    n_per_core: int = N_PER_CORE,
    reps: int = 1,
    nc: bass.Bass | None = None,
):
    import ml_dtypes

    if nc is None:
        nc = build(n_per_core, reps)
    ident = np.eye(128, dtype=ml_dtypes.bfloat16)
    ft_names = {
        a.memorylocations[0].name
        for a in nc.m.functions[0].allocations
        if getattr(a, "kind", None) == "ExternalInput"
    }
    in_maps = []
    for c in range(N_CORES):
        sl = slice(c * n_per_core, (c + 1) * n_per_core)
        m = {
            "a1": np.ascontiguousarray(a1[sl]),
            "a2": np.ascontiguousarray(a2[sl]).reshape(n_per_core, DH),
            "ident": ident,
        }
        if "ft" in ft_names:
            m["ft"] = np.ascontiguousarray(ft[sl]).reshape(n_per_core, ELEMS)
        in_maps.append(m)
    res = run_bass_kernel_spmd(nc, in_maps, list(range(N_CORES)))
    out = np.concatenate([res.results[c]["out"] for c in range(N_CORES)], axis=0)
    return np.asarray(out, dtype=np.float32).reshape(-1, H, F)


def kernel(a1: np.ndarray, a2: np.ndarray, ft: np.ndarray) -> np.ndarray:
    a1 = np.asarray(a1, dtype=np.float32)
    a2 = np.asarray(a2, dtype=np.float32)
    ft = np.asarray(ft, dtype=np.float32)
    assert a1.shape == (N, H, 1) and a2.shape == (N, D, H, 1)
    assert ft.shape == (N, D, H, F)
    out = run(a1.reshape(N, H), a2.reshape(N, D, H), ft)
    return out.astype(np.float32)
